# revision 1
# baseline (speedup 1.0000x reference)
"""Trainium2 Bass kernel for nn_DropGlobalScaledDotProductAttention.

Computation (reference semantics):
  a = d1 @ W1[:256]; c = d0 @ W1[256:]
  h[b,i,j,:] = relu(a[b,i,:] + c[b,j,:] + b1)          # [b,512,512,512]
  logits = h @ W2 + b2                                  # [b,512,512,2]
  drop[b,i,j] = argmax(logits) == 1  <=>  h @ (W2[:,1]-W2[:,0]) > b2[0]-b2[1]
  attn[b,n,i,j] = (q/8 . k) - 1e9 * drop[b,i,j]

Device strategy (8 cores, SPMD):
  Shard (batch, query-block): core c -> batch c//4, query rows [128*(c%4), ...).
  Per core, phase C streams 512 relu tiles T[f=128part, j=512] (bf16)
  produced by DVE (fused add+relu tensor_scalar, 4x mode) and ACT (Relu
  activation with per-partition bias), consumed by bf16 PE matmuls
  (1 cyc/row; a single dtype for every matmul in the stream -- mixing
  dtypes costs the PE ~60ns per switch) that reduce over f against
  w2d = W2[:,1]-W2[:,0].  To give each query row i its own PSUM row, the
  stationary operand is a shifted window of a zero matrix Z with w2d at
  column 32: lhsT = Z[:, 32-u : 64-u] puts w2d at column u, so query u's
  reduction lands in PSUM row u of a [32, 512] accumulating tile (other
  rows receive +0).

  The drop decision is sign(delta - t).  bf16 tiles give delta ~4e-3
  absolute error; decision margins can be as small as 3e-7.  The kernel
  therefore also outputs delta, and the host recomputes the few pairs with
  |delta - t| < TAU_FIX in float64 and patches the flipped decisions
  exactly (verified to reproduce the reference's fp32 decisions on all
  524288 pairs).
"""

import numpy as np

B, N, LQ, DK, DD = 2, 8, 512, 64, 256
F = 2 * DD          # 512 pairwise-MLP hidden dim
FC = F // 128       # 4 f-chunks
NCORES = 8
IBLK = LQ // 4      # 128 query rows per core
NEG = -1e9
TAU_FIX = 1.2e-2    # host-recompute band around the decision threshold

_CACHE = {}


def _build_nc():
    import concourse.bacc as bacc
    import concourse.tile as tile
    from concourse import mybir

    f32 = mybir.dt.float32
    f32r = mybir.dt.float32r
    bf16 = mybir.dt.bfloat16
    Alu = mybir.AluOpType
    Act = mybir.ActivationFunctionType

    nc = bacc.Bacc("TRN2", target_bir_lowering=False, debug=False,
                   num_devices=NCORES)

    # packA rows: w1b[2,512] | d0t[2,512] | w1a[2,512] | d1t[2,128]  (bf16)
    d_packA = nc.dram_tensor("packA", [128, 3328], bf16,
                             kind="ExternalInput").ap()
    d_b1c = nc.dram_tensor("b1c", [128, FC], f32, kind="ExternalInput").ap()
    d_w2cb = nc.dram_tensor("w2cb", [128, FC, 1], bf16, kind="ExternalInput").ap()
    d_qt = nc.dram_tensor("qt", [64, N, IBLK], f32, kind="ExternalInput").ap()
    d_kt = nc.dram_tensor("kt", [64, N, LQ], f32, kind="ExternalInput").ap()
    d_thr = nc.dram_tensor("thr", [128, 1], f32, kind="ExternalInput").ap()
    d_attn = nc.dram_tensor("attn", [N, IBLK, LQ], f32, kind="ExternalOutput").ap()
    d_delta = nc.dram_tensor("delta", [IBLK, LQ], f32, kind="ExternalOutput").ap()

    with tile.TileContext(nc) as tc:
        with (
            tc.tile_pool(name="const", bufs=1) as const,
            tc.tile_pool(name="tp", bufs=16) as tp,
            tc.tile_pool(name="op", bufs=4) as op,
            tc.tile_pool(name="ps", bufs=2, space="PSUM") as ps,
        ):
            # ---- loads (all host-prearranged into SBUF layouts) ----
            sb_packA = const.tile([128, 3328], bf16)
            sb_w1b = sb_packA[:, 0:1024].rearrange("p (c f) -> p c f", c=2)
            sb_d0t = sb_packA[:, 1024:2048].rearrange("p (c f) -> p c f", c=2)
            sb_w1a = sb_packA[:, 2048:3072].rearrange("p (c f) -> p c f", c=2)
            sb_d1t = sb_packA[:, 3072:3328].rearrange("p (c f) -> p c f", c=2)
            sb_b1 = const.tile([128, FC], f32)
            sb_w2zb = const.tile([128, FC, 64], bf16)
            sb_qt = const.tile([64, N, IBLK], f32)
            sb_kt = const.tile([64, N, LQ], f32)
            sb_thr = const.tile([128, 1], f32)
            # phase-A inputs first on the fast sync queue; q/k in background.
            # The Z windows are mostly zeros: memset + narrow DMA of the w2d
            # column instead of shipping 192KB of zeros.
            nc.vector.memset(sb_w2zb[:], 0.0)
            nc.sync.dma_start(out=sb_packA[:], in_=d_packA[:])
            nc.sync.dma_start(out=sb_b1[:], in_=d_b1c[:])
            nc.sync.dma_start(out=sb_w2zb[:, :, 32:33], in_=d_w2cb[:])
            nc.sync.dma_start(out=sb_thr[:], in_=d_thr[:])
            nc.gpsimd.dma_start(out=sb_qt[:], in_=d_qt[:])
            nc.gpsimd.dma_start(out=sb_kt[:], in_=d_kt[:])

            # ---- PE warmup during the input-DMA window: ~4us of dummy
            # matmuls flip the HAM to full clock so phase A runs warm.
            warm_x = const.tile([128, LQ], bf16)
            warm_w = const.tile([128, 32], bf16)
            nc.vector.memset(warm_x[:], 0.0)
            nc.vector.memset(warm_w[:], 0.0)
            pwu = ps.tile([32, LQ], f32, name="pwu", tag="pd")
            for t in range(10):
                nc.tensor.matmul(pwu[:], warm_w[:], warm_x[:],
                                 start=True, stop=True, skip_group_check=True)

            # ---- phase A: Ct[f,j] = (d0 @ W1b).T ; At[f,i] = (d1 @ W1a).T + b1
            # ct is kept in bf16: the DVE producer then runs in 4x mode
            # (bf16 in + bf16 out, both read ports packed).
            ct = []
            at = []
            for fc in range(FC):
                pa = ps.tile([128, LQ], f32, name="pa", tag="paq", bufs=5)
                for dc in range(2):
                    nc.tensor.matmul(
                        pa[:],
                        sb_w1b[:, dc, fc * 128:(fc + 1) * 128],
                        sb_d0t[:, dc, :],
                        start=(dc == 0), stop=(dc == 1),
                    )
                ct_fc = const.tile([128, LQ], bf16, name=f"ct{fc}", tag=f"ct{fc}")
                nc.vector.tensor_copy(ct_fc[:], pa[:])
                ct.append(ct_fc)
                pai = ps.tile([128, IBLK], f32, name="pai", tag="paq", bufs=5)
                for dc in range(2):
                    nc.tensor.matmul(
                        pai[:],
                        sb_w1a[:, dc, fc * 128:(fc + 1) * 128],
                        sb_d1t[:, dc, :],
                        start=(dc == 0), stop=(dc == 1),
                    )
                # 128B-aligned per-query bias columns (stride 32 floats):
                # misaligned scalar pointers cost the producers ~150ns/op
                at_fc = const.tile([128, IBLK, 32], f32, name=f"at{fc}",
                                   tag=f"at{fc}")
                nc.scalar.add(at_fc[:, :, 0], pai[:], sb_b1[:, fc:fc + 1])
                at.append(at_fc)

            # ---- phase C: delta[i, j] = sum_f w2d[f] relu(At[f,i] + Ct[f,j])
            # Query u of a 32-row group lands in PSUM row u via a shifted
            # stationary window (w2d at column u of Z).  DVE and ACT produce
            # the relu tiles; every matmul is bf16 (one dtype, no PE mode
            # switches).
            mask_full = const.tile([IBLK, LQ], f32)
            # producer rotation: V=vector (bf16 4x), A=scalar/ACT
            PAT = "AVVV" "AVVV" "AVVA" "VVVA"
            k = 0
            for g in range(IBLK // 32):
                pd = ps.tile([32, LQ], f32, name="pd", tag="pd")
                for u in range(32):
                    i = 32 * g + u
                    for fc in range(FC):
                        eng = PAT[k % 16]
                        k += 1
                        if eng == "A":
                            T = tp.tile([128, LQ], bf16, name="T", tag="T")
                            nc.scalar.activation(
                                T[:], ct[fc][:], Act.Relu,
                                bias=at[fc][:, i, 0:1], scale=1.0)
                            w = sb_w2zb
                        else:
                            T = tp.tile([128, LQ], bf16, name="Tb", tag="Tb")
                            nc.vector.tensor_scalar(
                                T[:], ct[fc][:], at[fc][:, i, 0:1], 0.0,
                                Alu.add, Alu.max)
                            w = sb_w2zb
                        nc.tensor.matmul(
                            pd[:],
                            w[:, fc, 32 - u:64 - u],
                            T[:],
                            start=(u == 0 and fc == 0),
                            stop=(u == 31 and fc == FC - 1),
                            skip_group_check=True,
                        )
                # mask rows = (delta > t) * NEG ; also export raw delta
                # mask reads PSUM directly so it doesn't serialize behind
                # the delta-export copy on the tail critical path
                nc.vector.tensor_scalar(
                    mask_full[32 * g:32 * g + 32, :], pd[:],
                    sb_thr[0:32, 0:1], NEG, Alu.is_gt, Alu.mult)
                delta_sb = op.tile([32, LQ], f32, name="delta_sb", tag="delta_sb")
                nc.scalar.copy(delta_sb[:], pd[:])
                nc.sync.dma_start(out=d_delta[32 * g:32 * g + 32, :],
                                  in_=delta_sb[:])

            # ---- phase D: attn[n] = qT[n].T @ kT[n] + mask
            for n in range(N):
                pq = ps.tile([IBLK, LQ], f32, name="pq", tag="paq", bufs=5)
                nc.tensor.matmul(pq[:], sb_qt[:, n, :], sb_kt[:, n, :],
                                 start=True, stop=True)
                out_t = op.tile([IBLK, LQ], f32, name="out_t", tag="out_t")
                nc.vector.tensor_add(out_t[:], pq[:], mask_full[:])
                nc.sync.dma_start(out=d_attn[n], in_=out_t[:])

    nc.compile()
    return nc


def _get_nc():
    if "nc" not in _CACHE:
        _CACHE["nc"] = _build_nc()
    return _CACHE["nc"]


def _prep_in_maps(q, k, d0, d1, W1, b1, W2, b2):
    f4 = np.float32
    import ml_dtypes

    bf = ml_dtypes.bfloat16
    w2d = (W2[:, 1] - W2[:, 0]).astype(f4)                    # [512]
    w2cb = np.ascontiguousarray(
        w2d.reshape(FC, 128).T.astype(f4))[:, :, None].astype(bf)  # [128,4,1]
    b1c = np.ascontiguousarray(b1.reshape(FC, 128).T.astype(f4))   # [128,4]
    w1a = W1[:DD].reshape(2, 128, F).transpose(1, 0, 2).astype(bf)  # [128,2,512]
    w1b = W1[DD:].reshape(2, 128, F).transpose(1, 0, 2).astype(bf)
    thr = np.full((128, 1), np.float32(b2[0]) - np.float32(b2[1]), dtype=f4)
    q8 = (q.astype(np.float64) / 8.0).astype(f4)              # exact (/8)

    in_maps = []
    for c in range(NCORES):
        b, blk = divmod(c, 4)
        isl = slice(blk * IBLK, (blk + 1) * IBLK)
        d1t = d1[b, isl, :].T.reshape(2, 128, IBLK).transpose(1, 0, 2).astype(bf)
        d0t = d0[b].T.reshape(2, 128, LQ).transpose(1, 0, 2).astype(bf)
        packA = np.ascontiguousarray(np.concatenate(
            [w1b.reshape(128, 1024), d0t.reshape(128, 1024),
             w1a.reshape(128, 1024), d1t.reshape(128, 256)], axis=1))
        qt = np.ascontiguousarray(q8[b, :, isl, :].transpose(2, 0, 1))  # [64,N,128]
        kt = np.ascontiguousarray(k[b].transpose(2, 0, 1))              # [64,N,512]
        in_maps.append({
            "packA": packA, "b1c": b1c, "w2cb": w2cb,
            "qt": qt, "kt": kt, "thr": thr,
        })
    return in_maps


def _host_fixup(attn, delta_dev, q, k, d0, d1, W1, b1, W2, b2):
    """Recompute decisions in float64 for pairs near the threshold and patch
    any flipped mask bits exactly."""
    f8 = np.float64
    d0_, d1_, W1_, b1_, W2_, b2_ = (x.astype(f8) for x in (d0, d1, W1, b1, W2, b2))
    w2d = W2_[:, 1] - W2_[:, 0]
    b2d = b2_[1] - b2_[0]
    thr = float(b2[0].astype(np.float32) - b2[1].astype(np.float32))

    a64 = np.einsum("bid,df->bif", d1_, W1_[:DD])
    c64 = np.einsum("bjd,df->bjf", d0_, W1_[DD:])

    border = np.argwhere(np.abs(delta_dev - thr) < TAU_FIX)
    nfix = 0
    for b, i, j in border:
        h = np.maximum(a64[b, i] + c64[b, j] + b1_, 0.0)
        want_drop = (h @ w2d + b2d) > 0.0
        dev_drop = delta_dev[b, i, j] > thr
        if want_drop != dev_drop:
            nfix += 1
            if want_drop:
                attn[b, :, i, j] = np.float32(NEG)
            else:
                qk = np.einsum("nd,nd->n", q[b, :, i, :].astype(f8) / 8.0,
                               k[b, :, j, :].astype(f8))
                attn[b, :, i, j] = qk.astype(np.float32)
    return len(border), nfix


def kernel(q, k, d0, d1, W1, b1, W2, b2):
    from concourse import bass_utils

    q, k, d0, d1, W1, b1, W2, b2 = (
        np.asarray(x) for x in (q, k, d0, d1, W1, b1, W2, b2))
    nc = _get_nc()
    in_maps = _prep_in_maps(q, k, d0, d1, W1, b1, W2, b2)
    res = bass_utils.run_bass_kernel_spmd(nc, in_maps, list(range(NCORES)))
    outs = res.results

    attn = np.empty((B, N, LQ, LQ), dtype=np.float32)
    delta = np.empty((B, LQ, LQ), dtype=np.float32)
    for c in range(NCORES):
        b, blk = divmod(c, 4)
        isl = slice(blk * IBLK, (blk + 1) * IBLK)
        attn[b, :, isl, :] = outs[c]["attn"]
        delta[b, isl, :] = outs[c]["delta"]

    _host_fixup(attn, delta, q, k, d0, d1, W1, b1, W2, b2)
    return attn

